# revision 14
# baseline (speedup 1.0000x reference)
"""DeepseekV3 MoE gate (moe_routing) for 8x TRN2 NeuronCores.

Sharding: data-parallel over tokens. Each core gets a 2048-token shard of x
(pre-transposed on host to [H, Tc] so both matmul operands DMA naturally with
the contraction dim on partitions); the small gate weight [7168, 256] and bias
are replicated.

Per-core pipeline, per 128-token tile:
  PE:   logits = xT_chunk.T @ W_chunk accumulated over 56 K-chunks in PSUM
  ACT:  s0 = sigmoid(logits)  (PSUM -> SBUF eviction fused)
  DVE:  b = s0 + bias; per-group Max8 -> top-2 sums -> group top-4 threshold
        -> additive mask; Max8/MaxIndex8 over masked scores -> top-8 experts;
        z = s0 + #{k: ms >= vals8[k]} (rank bands), Max8(z) recovers the
        selected s0 in exact rank order without any gather;
        normalize * 2.5, emit weights + indices.

Matmul runs as an fp16 hi/lo 3-term split (xh@Wh + xh@Wl + xl@Wh, split done
host-side) at 1 cycle/row on the PE with ~2e-7 logit error — fp32-grade
ranking fidelity at ~3.5x the speed of the PE's serialized fp32 mode.
"""

import sys

if "/opt/trn_rl_repo" not in sys.path:
    sys.path.insert(0, "/opt/trn_rl_repo")

from contextlib import ExitStack

import numpy as np

import concourse.bass as bass
import concourse.mybir as mybir
import concourse.tile as tile
from concourse import bacc
from concourse._compat import with_exitstack

H = 7168
E = 256
G = 8
EPG = E // G  # 32
K = 8
NEG = -1.0e30
ROUTE_SCALE = 2.5
P = 128

N_CORES = 8
T_FULL = 16384
T_CORE = T_FULL // N_CORES  # 2048

MODE = "f16x3"  # "f32r" | "f32" | "f16x3"
BLK = 512
KPACK_DEFAULT = 4
XBUFS = 8


def np_algo_reference(x, W, bias):
    """Numpy mirror of the kernel algorithm (for validation in tests)."""
    x = x.astype(np.float32)
    T = x.shape[0]
    logits = (x.astype(np.float64) @ W.astype(np.float64)).astype(np.float32)
    s0 = (1.0 / (1.0 + np.exp(-logits.astype(np.float64)))).astype(np.float32)
    b = s0 + bias.astype(np.float32)
    bg = b.reshape(T, G, EPG)
    top2 = np.sort(bg, axis=-1)[:, :, -2:]
    gs = (top2[:, :, 0] + top2[:, :, 1]).astype(np.float32)
    gsort = np.sort(gs, axis=-1)[:, ::-1]
    thresh = gsort[:, 3:4]
    pen = np.where(gs >= thresh, np.float32(0.0), np.float32(NEG))
    ms = b + np.repeat(pen, EPG, axis=1)
    order = np.argsort(-ms, axis=-1, kind="stable")[:, :K]
    s0sel = np.take_along_axis(s0, order, axis=-1)
    q = np.arange(K, 0, -1).astype(np.float32)
    z = (q[None, :] + s0sel).astype(np.float32)
    s0sel_rt = (z - q[None, :]).astype(np.float32)
    ssum = s0sel_rt.sum(-1, keepdims=True, dtype=np.float32)
    wts = (s0sel_rt * ((np.float32(1.0) / ssum) * np.float32(ROUTE_SCALE))).astype(
        np.float32
    )
    return wts, order.astype(np.int32)


@with_exitstack
def _gate_kernel(
    ctx: ExitStack,
    tc: tile.TileContext,
    outs,
    ins,
    T_core: int,
    BLK: int = 512,
    mode: str = "f32r",
    repeat: int = 1,
    taper: bool = False,
):
    nc = tc.nc
    wts_d, sel_d = outs
    if mode == "f16x3":
        xh_d, xl_d, wh_d, wl_d, bias_d = ins
    elif mode == "fp8x3":
        xa_d, xb_d, w_d, bias_d = ins
    else:
        xT_d, w_d, bias_d = ins

    n_k = H // P  # 56
    KPACK = KPACK_DEFAULT  # k-chunks per x DMA (fewer, larger DMAs)
    assert n_k % KPACK == 0
    n_tiles = T_core // P

    # Uniform block schedule (HW-verified configuration). With taper=True the
    # final blocks shrink (512->256->128->128) so the post-matmul routing
    # tail drains one tile deep instead of four.
    if taper:
        blocks = []
        t = 0
        rem = T_core
        while rem > 0:
            if rem > BLK:
                bs = BLK
            elif rem == BLK and BLK >= 4 * P:
                bs = BLK // 2
            elif rem > 2 * P:
                bs = rem - 2 * P
            else:
                bs = P
            bs = min(bs, rem)
            blocks.append((t, bs))
            t += bs
            rem -= bs
    else:
        blocks = [(i * BLK, BLK) for i in range(T_core // BLK)]

    f32 = mybir.dt.float32
    f16 = mybir.dt.float16
    assert T_core % BLK == 0 and BLK % P == 0

    const = ctx.enter_context(tc.tile_pool(name="const", bufs=1))
    wpool = ctx.enter_context(tc.tile_pool(name="wpool", bufs=1))
    xpool = ctx.enter_context(tc.tile_pool(name="xpool", bufs=XBUFS))
    ppool = ctx.enter_context(tc.tile_pool(name="ppool", bufs=8, space="PSUM"))
    spool = ctx.enter_context(tc.tile_pool(name="spool", bufs=3))
    opool = ctx.enter_context(tc.tile_pool(name="opool", bufs=3))

    # ---- constants ----
    # bias rides the DVE queue so the x stream owns the SP/HWDGE ring
    bias_bc = const.tile([P, E], f32)
    nc.scalar.dma_start(bias_bc[:], bias_d.unsqueeze(0).to_broadcast([P, E]))

    qrow32 = const.tile([P, K], f32)
    for k in range(K):
        nc.vector.memset(qrow32[:, k : k + 1], float(K - k))

    # output accumulators: one SBUF row-block per 128-token tile, DMA'd once
    outw_acc = const.tile([P, n_tiles, K], f32)
    outs_acc = const.tile([P, n_tiles, K], mybir.dt.int32)

    # ---- resident weights ----
    # Streamed on the HWDGE queue in KPACK-sized pieces, issued interleaved
    # with block 0's x stream below so the PE starts after one piece of each
    # instead of after the whole 7MB W load.
    if mode == "f16x3":
        w_all_h = wpool.tile([P, n_k, E], f16)
        w_all_l = wpool.tile([P, n_k, E], f16)
        wh_view = wh_d.rearrange("(k p) e -> p k e", p=P)
        wl_view = wl_d.rearrange("(k p) e -> p k e", p=P)

        def load_w_piece(k0, k1):
            nc.sync.dma_start(w_all_h[:, k0:k1, :], wh_view[:, k0:k1, :])
            nc.sync.dma_start(w_all_l[:, k0:k1, :], wl_view[:, k0:k1, :])
    elif mode == "fp8x3":
        # resident W levels, plane-interleaved (w0, w1, w2) per k-chunk so any
        # adjacent level pair is one strided AP
        f8 = mybir.dt.float8e4
        w_all = wpool.tile([P, n_k, 3, E], f8)
        w_view = w_d.rearrange("(k p) three e -> p k three e", p=P)

        def load_w_piece(k0, k1):
            nc.sync.dma_start(w_all[:, k0:k1, :, :], w_view[:, k0:k1, :, :])
    else:
        mdt = mybir.dt.float32r if mode == "f32r" else f32
        w_all = wpool.tile([P, n_k, E], mdt)
        w_view = w_d.rearrange("(k p) e -> p k e", p=P)

        def load_w_piece(k0, k1):
            nc.sync.dma_start(w_all[:, k0:k1, :], w_view[:, k0:k1, :])

    w_loaded = 0  # k-chunks of W issued so far

    # ---- main loop ----
    for rep, (tb, (t0, bs)) in [
        (r, b) for r in range(repeat) for b in enumerate(blocks)
    ]:
        n_sub = bs // P
        psums = []
        for s in range(n_sub):
            pt = ppool.tile([P, E], f32, name=f"psum_{rep}_{tb}_{s}", tag="psum")
            psums.append(pt)

        for k0 in range(0, n_k, KPACK):
            if w_loaded < n_k:
                # lazily stream the next W piece just ahead of its first use
                load_w_piece(w_loaded, min(w_loaded + KPACK, n_k))
                w_loaded = min(w_loaded + KPACK, n_k)
            if mode == "f16x3":
                xch = xpool.tile([P, KPACK, bs], f16, tag="xch")
                xcl = xpool.tile([P, KPACK, bs], f16, tag="xcl")
                nc.sync.dma_start(
                    xch[:],
                    xh_d[k0 * P : (k0 + KPACK) * P, t0 : t0 + bs].rearrange(
                        "(kk p) t -> p kk t", p=P
                    ),
                )
                nc.sync.dma_start(
                    xcl[:],
                    xl_d[k0 * P : (k0 + KPACK) * P, t0 : t0 + bs].rearrange(
                        "(kk p) t -> p kk t", p=P
                    ),
                )
                for kk in range(KPACK):
                    k = k0 + kk
                    start = k == 0
                    stop = k == n_k - 1
                    for s in range(n_sub):
                        lh = xch[:, kk, s * P : (s + 1) * P]
                        ll = xcl[:, kk, s * P : (s + 1) * P]
                        # xh stationary twice in a row -> cheaper weight reload
                        nc.tensor.matmul(
                            psums[s][:], lh, w_all_h[:, k, :], start=start, stop=False
                        )
                        nc.tensor.matmul(
                            psums[s][:], lh, w_all_l[:, k, :], start=False, stop=False
                        )
                        nc.tensor.matmul(
                            psums[s][:], ll, w_all_h[:, k, :], start=False, stop=stop
                        )
            else:
                xc = xpool.tile([P, KPACK, bs], mdt, tag="xch")
                nc.sync.dma_start(
                    xc[:],
                    xT_d[k0 * P : (k0 + KPACK) * P, t0 : t0 + bs].rearrange(
                        "(kk p) t -> p kk t", p=P
                    ),
                )
                for kk in range(KPACK):
                    k = k0 + kk
                    for s in range(n_sub):
                        nc.tensor.matmul(
                            psums[s][:],
                            xc[:, kk, s * P : (s + 1) * P],
                            w_all[:, k, :],
                            start=(k == 0),
                            stop=(k == n_k - 1),
                        )

        for s in range(n_sub):
            trow = t0 + s * P
            s0 = spool.tile([P, E], f32, tag="s0")
            nc.scalar.activation(
                s0[:], psums[s][:], mybir.ActivationFunctionType.Sigmoid
            )
            b = spool.tile([P, E], f32, tag="b")
            nc.vector.tensor_add(b[:], s0[:], bias_bc[:])
            gmax = opool.tile([P, G * 8], f32, tag="gmax")
            for g in range(G):
                nc.vector.max(
                    out=gmax[:, g * 8 : (g + 1) * 8],
                    in_=b[:, g * EPG : (g + 1) * EPG],
                )
            gv = gmax[:].rearrange("p (g c) -> p g c", g=G)
            gs = opool.tile([P, G], f32, tag="gs")
            nc.vector.tensor_add(gs[:], gv[:, :, 0], gv[:, :, 1])
            gtop = opool.tile([P, 8], f32, tag="gtop")
            nc.vector.max(out=gtop[:], in_=gs[:])
            pen = opool.tile([P, G], f32, tag="pen")
            nc.vector.tensor_scalar(
                pen[:],
                gs[:],
                gtop[:, 3:4],
                None,
                op0=mybir.AluOpType.is_ge,
            )
            nc.vector.tensor_scalar(
                pen[:],
                pen[:],
                1.0,
                -NEG,
                op0=mybir.AluOpType.subtract,
                op1=mybir.AluOpType.mult,
            )
            ms = spool.tile([P, E], f32, tag="ms")
            pen_bc = pen[:].unsqueeze(2).to_broadcast([P, G, EPG])
            nc.vector.tensor_add(
                ms[:].rearrange("p (g c) -> p g c", g=G),
                b[:].rearrange("p (g c) -> p g c", g=G),
                pen_bc,
            )
            vals8 = opool.tile([P, K], f32, tag="vals8")
            nc.vector.max(out=vals8[:], in_=ms[:])
            idxu = opool.tile([P, K], mybir.dt.uint16, tag="idxu")
            nc.vector.max_index(idxu[:], vals8[:], ms[:])
            # z[p,e] = s0[p,e] + #{k : ms[p,e] >= vals8[p,k]}
            # selected rank-r expert lands in band (8-r, 9-r); others in (0,1)
            z = spool.tile([P, E], f32, tag="z")
            nc.vector.scalar_tensor_tensor(
                z[:],
                ms[:],
                vals8[:, 0:1],
                s0[:],
                op0=mybir.AluOpType.is_ge,
                op1=mybir.AluOpType.add,
            )
            for k in range(1, K):
                nc.vector.scalar_tensor_tensor(
                    z[:],
                    ms[:],
                    vals8[:, k : k + 1],
                    z[:],
                    op0=mybir.AluOpType.is_ge,
                    op1=mybir.AluOpType.add,
                )
            zv = opool.tile([P, K], f32, tag="zv")
            nc.vector.max(out=zv[:], in_=z[:])
            s0sel = opool.tile([P, K], f32, tag="s0sel")
            nc.vector.tensor_sub(s0sel[:], zv[:], qrow32[:])
            ssum = opool.tile([P, 1], f32, tag="ssum")
            nc.vector.tensor_reduce(
                ssum[:], s0sel[:], axis=mybir.AxisListType.X, op=mybir.AluOpType.add
            )
            rec = opool.tile([P, 1], f32, tag="rec")
            nc.vector.reciprocal(rec[:], ssum[:])
            ti = trow // P
            nc.vector.tensor_scalar(
                outw_acc[:, ti, :],
                s0sel[:],
                rec[:],
                ROUTE_SCALE,
                op0=mybir.AluOpType.mult,
                op1=mybir.AluOpType.mult,
            )
            nc.vector.tensor_copy(outs_acc[:, ti, :], idxu[:])

        # flush this block's outputs so only the last block's tail is exposed.
        # DRAM layout is [P, n_tiles*K] (token-tile-major per partition) so
        # each partition line is one contiguous run; host unshuffles for free.
        # flushed from the DVE queue: keeps the SP/HWDGE ring free for x, so
        # a flush waiting on routing can't head-of-line-block the next block's
        # x prefetches.
        ti0 = t0 // P
        nc.scalar.dma_start(
            wts_d.rearrange("p (t k) -> p t k", k=K)[:, ti0 : ti0 + n_sub, :],
            outw_acc[:, ti0 : ti0 + n_sub, :],
        )
        nc.scalar.dma_start(
            sel_d.rearrange("p (t k) -> p t k", k=K)[:, ti0 : ti0 + n_sub, :],
            outs_acc[:, ti0 : ti0 + n_sub, :],
        )


_NC_CACHE = {}


TAPER = False


def _build(mode=MODE, t_core=T_CORE, blk=BLK, repeat=1, taper=None):
    if taper is None:
        taper = TAPER
    key = (mode, t_core, blk, repeat, taper)
    if key in _NC_CACHE:
        return _NC_CACHE[key]
    nc = bacc.Bacc("TRN2", target_bir_lowering=False, debug=False)
    f32 = mybir.dt.float32
    f16 = mybir.dt.float16
    if mode == "f16x3":
        ins = [
            nc.dram_tensor("xh", [H, t_core], f16, kind="ExternalInput").ap(),
            nc.dram_tensor("xl", [H, t_core], f16, kind="ExternalInput").ap(),
            nc.dram_tensor("wh", [H, E], f16, kind="ExternalInput").ap(),
            nc.dram_tensor("wl", [H, E], f16, kind="ExternalInput").ap(),
            nc.dram_tensor("bias", [E], f32, kind="ExternalInput").ap(),
        ]
    else:
        mdt = mybir.dt.float32r if mode == "f32r" else f32
        ins = [
            nc.dram_tensor("xT", [H, t_core], mdt, kind="ExternalInput").ap(),
            nc.dram_tensor("w", [H, E], mdt, kind="ExternalInput").ap(),
            nc.dram_tensor("bias", [E], f32, kind="ExternalInput").ap(),
        ]
    n_tiles = t_core // P
    outs = [
        nc.dram_tensor("wts", [P, n_tiles * K], f32, kind="ExternalOutput").ap(),
        nc.dram_tensor(
            "sel", [P, n_tiles * K], mybir.dt.int32, kind="ExternalOutput"
        ).ap(),
    ]
    with tile.TileContext(nc) as tc:
        _gate_kernel(
            tc, outs, ins, T_core=t_core, BLK=blk, mode=mode, repeat=repeat,
            taper=taper,
        )
    nc.compile()
    _NC_CACHE[key] = nc
    return nc


def _make_in_maps(x, W_gate, bias, mode=MODE):
    x = np.asarray(x, dtype=np.float32)
    W_gate = np.asarray(W_gate, dtype=np.float32)
    bias = np.asarray(bias, dtype=np.float32)
    in_maps = []
    if mode == "f16x3":
        Wh = W_gate.astype(np.float16)
        Wl = (W_gate - Wh.astype(np.float32)).astype(np.float16)
        for c in range(N_CORES):
            xT = x[c * T_CORE : (c + 1) * T_CORE].T
            xh = np.ascontiguousarray(xT.astype(np.float16))
            xl = np.ascontiguousarray(
                (xT - xh.astype(np.float32)).astype(np.float16)
            )
            in_maps.append({"xh": xh, "xl": xl, "wh": Wh, "wl": Wl, "bias": bias})
    else:
        for c in range(N_CORES):
            xT = np.ascontiguousarray(x[c * T_CORE : (c + 1) * T_CORE].T)
            in_maps.append({"xT": xT, "w": W_gate, "bias": bias})
    return in_maps


_NEFF_CACHE_DIR = "/tmp/bass_neff_cache"
_neff_cache_installed = False


def _install_neff_cache():
    """Cache compiled NEFFs by BIR hash so repeat runs skip walrus."""
    global _neff_cache_installed
    if _neff_cache_installed:
        return
    import hashlib
    import os
    import shutil

    from concourse import bass2jax, bass_utils

    orig = bass_utils.compile_bir_kernel

    def cached(bir_json, tmpdir, neff_name="file.neff"):
        h = hashlib.sha256(bir_json).hexdigest()[:24]
        os.makedirs(_NEFF_CACHE_DIR, exist_ok=True)
        cpath = os.path.join(_NEFF_CACHE_DIR, h + ".neff")
        out = os.path.join(tmpdir, neff_name)
        if os.path.exists(cpath):
            shutil.copy(cpath, out)
            return out
        p = orig(bir_json, tmpdir, neff_name)
        try:
            shutil.copy(p, cpath)
        except OSError:
            pass
        return p

    bass2jax.compile_bir_kernel = cached
    _neff_cache_installed = True


def run_on_hw(x, W_gate, bias, mode=MODE, trace=False, **kwargs):
    from concourse import bass_utils

    _install_neff_cache()
    nc = _build(mode)
    in_maps = _make_in_maps(x, W_gate, bias, mode)
    res = bass_utils.run_bass_kernel_spmd(
        nc, in_maps, list(range(N_CORES)), trace=trace, **kwargs
    )

    def unshuffle(a):
        # device layout [P, n_tiles*K]: row p, tile t -> token t*P + p
        a = np.asarray(a).reshape(P, T_CORE // P, K)
        return np.ascontiguousarray(a.transpose(1, 0, 2).reshape(T_CORE, K))

    wts = np.concatenate([unshuffle(r["wts"]) for r in res.results], axis=0)
    sel = np.concatenate([unshuffle(r["sel"]) for r in res.results], axis=0)
    return (wts.astype(np.float32), sel.astype(np.int32)), res


def kernel(x, W_gate, bias):
    (wts, sel), _ = run_on_hw(x, W_gate, bias, MODE)
    return wts, sel



# revision 17
# speedup vs baseline: 1.0640x; 1.0640x over previous
"""DeepseekV3 MoE gate (moe_routing) for 8x TRN2 NeuronCores.

Sharding: data-parallel over tokens. Each core gets a 2048-token shard of x
(pre-transposed on host to [H, Tc] so both matmul operands DMA naturally with
the contraction dim on partitions); the small gate weight [7168, 256] and bias
are replicated.

Per-core pipeline, per 128-token tile:
  PE:   logits = xT_chunk.T @ W_chunk accumulated over 56 K-chunks in PSUM
  ACT:  s0 = sigmoid(logits)  (PSUM -> SBUF eviction fused)
  DVE:  b = s0 + bias; per-group Max8 -> top-2 sums -> group top-4 threshold
        -> additive mask; Max8/MaxIndex8 over masked scores -> top-8 experts;
        z = s0 + #{k: ms >= vals8[k]} (rank bands), Max8(z) recovers the
        selected s0 in exact rank order without any gather;
        normalize * 2.5, emit weights + indices.

Matmul runs as an fp16 hi/lo 3-term split (xh@Wh + xh@Wl + xl@Wh, split done
host-side) at 1 cycle/row on the PE with ~2e-7 logit error — fp32-grade
ranking fidelity at ~3.5x the speed of the PE's serialized fp32 mode.
"""

import sys

if "/opt/trn_rl_repo" not in sys.path:
    sys.path.insert(0, "/opt/trn_rl_repo")

from contextlib import ExitStack

import numpy as np

import concourse.bass as bass
import concourse.mybir as mybir
import concourse.tile as tile
from concourse import bacc
from concourse._compat import with_exitstack

H = 7168
E = 256
G = 8
EPG = E // G  # 32
K = 8
NEG = -1.0e30
ROUTE_SCALE = 2.5
P = 128

N_CORES = 8
T_FULL = 16384
T_CORE = T_FULL // N_CORES  # 2048

MODE = "f16x3"  # "f32r" | "f32" | "f16x3"
BLK = 512
KPACK_DEFAULT = 4
XBUFS = 8


def np_algo_reference(x, W, bias):
    """Numpy mirror of the kernel algorithm (for validation in tests)."""
    x = x.astype(np.float32)
    T = x.shape[0]
    logits = (x.astype(np.float64) @ W.astype(np.float64)).astype(np.float32)
    s0 = (1.0 / (1.0 + np.exp(-logits.astype(np.float64)))).astype(np.float32)
    b = s0 + bias.astype(np.float32)
    bg = b.reshape(T, G, EPG)
    top2 = np.sort(bg, axis=-1)[:, :, -2:]
    gs = (top2[:, :, 0] + top2[:, :, 1]).astype(np.float32)
    gsort = np.sort(gs, axis=-1)[:, ::-1]
    thresh = gsort[:, 3:4]
    pen = np.where(gs >= thresh, np.float32(0.0), np.float32(NEG))
    ms = b + np.repeat(pen, EPG, axis=1)
    order = np.argsort(-ms, axis=-1, kind="stable")[:, :K]
    s0sel = np.take_along_axis(s0, order, axis=-1)
    q = np.arange(K, 0, -1).astype(np.float32)
    z = (q[None, :] + s0sel).astype(np.float32)
    s0sel_rt = (z - q[None, :]).astype(np.float32)
    ssum = s0sel_rt.sum(-1, keepdims=True, dtype=np.float32)
    wts = (s0sel_rt * ((np.float32(1.0) / ssum) * np.float32(ROUTE_SCALE))).astype(
        np.float32
    )
    return wts, order.astype(np.int32)


@with_exitstack
def _gate_kernel(
    ctx: ExitStack,
    tc: tile.TileContext,
    outs,
    ins,
    T_core: int,
    BLK: int = 512,
    mode: str = "f32r",
    repeat: int = 1,
    taper: bool = False,
):
    nc = tc.nc
    wts_d, sel_d = outs
    if mode == "f16x3":
        xh_d, xl_d, wh_d, wl_d, bias_d = ins
    elif mode == "fp8x3":
        xa_d, xb_d, w_d, bias_d = ins
    else:
        xT_d, w_d, bias_d = ins

    n_k = H // P  # 56
    KPACK = KPACK_DEFAULT  # k-chunks per x DMA (fewer, larger DMAs)
    assert n_k % KPACK == 0
    n_tiles = T_core // P

    # Uniform block schedule (HW-verified configuration). With taper=True the
    # final blocks shrink (512->256->128->128) so the post-matmul routing
    # tail drains one tile deep instead of four.
    if taper:
        blocks = []
        t = 0
        rem = T_core
        while rem > 0:
            if rem > BLK:
                bs = BLK
            elif rem == BLK and BLK >= 4 * P:
                bs = BLK // 2
            elif rem > 2 * P:
                bs = rem - 2 * P
            else:
                bs = P
            bs = min(bs, rem)
            blocks.append((t, bs))
            t += bs
            rem -= bs
    else:
        blocks = [(i * BLK, BLK) for i in range(T_core // BLK)]

    f32 = mybir.dt.float32
    f16 = mybir.dt.float16
    assert T_core % BLK == 0 and BLK % P == 0

    const = ctx.enter_context(tc.tile_pool(name="const", bufs=1))
    wpool = ctx.enter_context(tc.tile_pool(name="wpool", bufs=1))
    xpool = ctx.enter_context(tc.tile_pool(name="xpool", bufs=XBUFS))
    ppool = ctx.enter_context(tc.tile_pool(name="ppool", bufs=8, space="PSUM"))
    spool = ctx.enter_context(tc.tile_pool(name="spool", bufs=3))
    opool = ctx.enter_context(tc.tile_pool(name="opool", bufs=3))

    # ---- constants ----
    # bias rides the DVE queue so the x stream owns the SP/HWDGE ring
    bias_bc = const.tile([P, E], f32)
    nc.scalar.dma_start(bias_bc[:], bias_d.unsqueeze(0).to_broadcast([P, E]))

    qrow32 = const.tile([P, K], f32)
    for k in range(K):
        nc.vector.memset(qrow32[:, k : k + 1], float(K - k))

    # output accumulators: one SBUF row-block per 128-token tile, DMA'd once
    outw_acc = const.tile([P, n_tiles, K], f32)
    outs_acc = const.tile([P, n_tiles, K], mybir.dt.int32)

    # ---- resident weights ----
    # Streamed on the HWDGE queue in KPACK-sized pieces, issued interleaved
    # with block 0's x stream below so the PE starts after one piece of each
    # instead of after the whole 7MB W load.
    if mode == "f16x3":
        w_all_h = wpool.tile([P, n_k, E], f16)
        w_all_l = wpool.tile([P, n_k, E], f16)
        wh_view = wh_d.rearrange("(k p) e -> p k e", p=P)
        wl_view = wl_d.rearrange("(k p) e -> p k e", p=P)

        def load_w_piece(k0, k1):
            nc.sync.dma_start(w_all_h[:, k0:k1, :], wh_view[:, k0:k1, :])
            nc.sync.dma_start(w_all_l[:, k0:k1, :], wl_view[:, k0:k1, :])
    elif mode == "fp8x3":
        # resident W levels, plane-interleaved (w0, w1, w2) per k-chunk so any
        # adjacent level pair is one strided AP
        f8 = mybir.dt.float8e4
        w_all = wpool.tile([P, n_k, 3, E], f8)
        w_view = w_d.rearrange("(k p) three e -> p k three e", p=P)

        def load_w_piece(k0, k1):
            nc.sync.dma_start(w_all[:, k0:k1, :, :], w_view[:, k0:k1, :, :])
    else:
        mdt = mybir.dt.float32r if mode == "f32r" else f32
        w_all = wpool.tile([P, n_k, E], mdt)
        w_view = w_d.rearrange("(k p) e -> p k e", p=P)

        def load_w_piece(k0, k1):
            nc.sync.dma_start(w_all[:, k0:k1, :], w_view[:, k0:k1, :])

    w_loaded = 0  # k-chunks of W issued so far

    # ---- main loop ----
    for rep, (tb, (t0, bs)) in [
        (r, b) for r in range(repeat) for b in enumerate(blocks)
    ]:
        n_sub = bs // P
        if mode == "fp8x3":
            psums3 = []
            for s in range(n_sub):
                p0 = ppool.tile([P, E], f32, name=f"psum0_{rep}_{tb}_{s}", tag="psum")
                p6 = ppool.tile([P, E], f32, name=f"psum6_{rep}_{tb}_{s}", tag="psum")
                p12 = ppool.tile(
                    [P, E], f32, name=f"psum12_{rep}_{tb}_{s}", tag="psum"
                )
                psums3.append((p0, p6, p12))
        else:
            psums = []
            for s in range(n_sub):
                pt = ppool.tile([P, E], f32, name=f"psum_{rep}_{tb}_{s}", tag="psum")
                psums.append(pt)

        for k0 in range(0, n_k, KPACK):
            if w_loaded < n_k:
                # lazily stream the next W piece just ahead of its first use
                load_w_piece(w_loaded, min(w_loaded + KPACK, n_k))
                w_loaded = min(w_loaded + KPACK, n_k)
            if mode == "fp8x3":
                f8 = mybir.dt.float8e4
                DR = mybir.MatmulPerfMode.DoubleRow
                xa = xpool.tile([P, KPACK, 2, bs], f8, tag="xa")
                xb = xpool.tile([P, KPACK, bs], f8, tag="xb")
                nc.sync.dma_start(
                    xa[:],
                    xa_d[k0 * P : (k0 + KPACK) * P, :, t0 : t0 + bs].rearrange(
                        "(kk p) two t -> p kk two t", p=P
                    ),
                )
                nc.sync.dma_start(
                    xb[:],
                    xb_d[k0 * P : (k0 + KPACK) * P, t0 : t0 + bs].rearrange(
                        "(kk p) t -> p kk t", p=P
                    ),
                )
                # 6 level-products per chunk as 3 DoubleRow matmuls:
                #   P6  += x1[k]@w0[k] + x0[k]@w1[k]     (planes within chunk)
                #   P12 += x1[k]@w1[k] + x0[k]@w2[k]     (planes within chunk)
                #   P0  += x0[k]@w0[k] + x0[k+1]@w0[k+1] (chunk-paired)
                #   P12 += x2[k]@w0[k] + x2[k+1]@w0[k+1] (chunk-paired)
                # xa planes are (x1, x0); w planes are (w0, w1, w2).
                for kk in range(KPACK):
                    k = k0 + kk
                    for s in range(n_sub):
                        ts_ = slice(s * P, (s + 1) * P)
                        p0, p6, p12 = psums3[s]
                        nc.tensor.matmul(
                            p6[:],
                            xa[:, kk, :, ts_],
                            w_all[:, k, 0:2, :],
                            start=(k == 0),
                            stop=(k == n_k - 1),
                            perf_mode=DR,
                        )
                        nc.tensor.matmul(
                            p12[:],
                            xa[:, kk, :, ts_],
                            w_all[:, k, 1:3, :],
                            start=(k == 0),
                            stop=False,
                            perf_mode=DR,
                        )
                        if k % 2 == 1:
                            nc.tensor.matmul(
                                p0[:],
                                xa[:, kk - 1 : kk + 1, 1, ts_],
                                w_all[:, k - 1 : k + 1, 0, :],
                                start=(k == 1),
                                stop=(k == n_k - 1),
                                perf_mode=DR,
                            )
                            nc.tensor.matmul(
                                p12[:],
                                xb[:, kk - 1 : kk + 1, ts_],
                                w_all[:, k - 1 : k + 1, 0, :],
                                start=False,
                                stop=(k == n_k - 1),
                                perf_mode=DR,
                            )
            elif mode == "f16x3":
                xch = xpool.tile([P, KPACK, bs], f16, tag="xch")
                xcl = xpool.tile([P, KPACK, bs], f16, tag="xcl")
                nc.sync.dma_start(
                    xch[:],
                    xh_d[k0 * P : (k0 + KPACK) * P, t0 : t0 + bs].rearrange(
                        "(kk p) t -> p kk t", p=P
                    ),
                )
                nc.sync.dma_start(
                    xcl[:],
                    xl_d[k0 * P : (k0 + KPACK) * P, t0 : t0 + bs].rearrange(
                        "(kk p) t -> p kk t", p=P
                    ),
                )
                for kk in range(KPACK):
                    k = k0 + kk
                    start = k == 0
                    stop = k == n_k - 1
                    for s in range(n_sub):
                        lh = xch[:, kk, s * P : (s + 1) * P]
                        ll = xcl[:, kk, s * P : (s + 1) * P]
                        # xh stationary twice in a row -> cheaper weight reload
                        nc.tensor.matmul(
                            psums[s][:], lh, w_all_h[:, k, :], start=start, stop=False
                        )
                        nc.tensor.matmul(
                            psums[s][:], lh, w_all_l[:, k, :], start=False, stop=False
                        )
                        nc.tensor.matmul(
                            psums[s][:], ll, w_all_h[:, k, :], start=False, stop=stop
                        )
            else:
                xc = xpool.tile([P, KPACK, bs], mdt, tag="xch")
                nc.sync.dma_start(
                    xc[:],
                    xT_d[k0 * P : (k0 + KPACK) * P, t0 : t0 + bs].rearrange(
                        "(kk p) t -> p kk t", p=P
                    ),
                )
                for kk in range(KPACK):
                    k = k0 + kk
                    for s in range(n_sub):
                        nc.tensor.matmul(
                            psums[s][:],
                            xc[:, kk, s * P : (s + 1) * P],
                            w_all[:, k, :],
                            start=(k == 0),
                            stop=(k == n_k - 1),
                        )

        for s in range(n_sub):
            trow = t0 + s * P
            s0 = spool.tile([P, E], f32, tag="s0")
            nc.scalar.activation(
                s0[:], psums[s][:], mybir.ActivationFunctionType.Sigmoid
            )
            b = spool.tile([P, E], f32, tag="b")
            nc.vector.tensor_add(b[:], s0[:], bias_bc[:])
            gmax = opool.tile([P, G * 8], f32, tag="gmax")
            for g in range(G):
                nc.vector.max(
                    out=gmax[:, g * 8 : (g + 1) * 8],
                    in_=b[:, g * EPG : (g + 1) * EPG],
                )
            gv = gmax[:].rearrange("p (g c) -> p g c", g=G)
            gs = opool.tile([P, G], f32, tag="gs")
            nc.vector.tensor_add(gs[:], gv[:, :, 0], gv[:, :, 1])
            gtop = opool.tile([P, 8], f32, tag="gtop")
            nc.vector.max(out=gtop[:], in_=gs[:])
            pen = opool.tile([P, G], f32, tag="pen")
            nc.vector.tensor_scalar(
                pen[:],
                gs[:],
                gtop[:, 3:4],
                None,
                op0=mybir.AluOpType.is_ge,
            )
            nc.vector.tensor_scalar(
                pen[:],
                pen[:],
                1.0,
                -NEG,
                op0=mybir.AluOpType.subtract,
                op1=mybir.AluOpType.mult,
            )
            ms = spool.tile([P, E], f32, tag="ms")
            pen_bc = pen[:].unsqueeze(2).to_broadcast([P, G, EPG])
            nc.vector.tensor_add(
                ms[:].rearrange("p (g c) -> p g c", g=G),
                b[:].rearrange("p (g c) -> p g c", g=G),
                pen_bc,
            )
            vals8 = opool.tile([P, K], f32, tag="vals8")
            nc.vector.max(out=vals8[:], in_=ms[:])
            idxu = opool.tile([P, K], mybir.dt.uint16, tag="idxu")
            nc.vector.max_index(idxu[:], vals8[:], ms[:])
            # z[p,e] = s0[p,e] + #{k : ms[p,e] >= vals8[p,k]}
            # selected rank-r expert lands in band (8-r, 9-r); others in (0,1)
            z = spool.tile([P, E], f32, tag="z")
            nc.vector.scalar_tensor_tensor(
                z[:],
                ms[:],
                vals8[:, 0:1],
                s0[:],
                op0=mybir.AluOpType.is_ge,
                op1=mybir.AluOpType.add,
            )
            for k in range(1, K):
                nc.vector.scalar_tensor_tensor(
                    z[:],
                    ms[:],
                    vals8[:, k : k + 1],
                    z[:],
                    op0=mybir.AluOpType.is_ge,
                    op1=mybir.AluOpType.add,
                )
            zv = opool.tile([P, K], f32, tag="zv")
            nc.vector.max(out=zv[:], in_=z[:])
            s0sel = opool.tile([P, K], f32, tag="s0sel")
            nc.vector.tensor_sub(s0sel[:], zv[:], qrow32[:])
            ssum = opool.tile([P, 1], f32, tag="ssum")
            nc.vector.tensor_reduce(
                ssum[:], s0sel[:], axis=mybir.AxisListType.X, op=mybir.AluOpType.add
            )
            rec = opool.tile([P, 1], f32, tag="rec")
            nc.vector.reciprocal(rec[:], ssum[:])
            ti = trow // P
            nc.vector.tensor_scalar(
                outw_acc[:, ti, :],
                s0sel[:],
                rec[:],
                ROUTE_SCALE,
                op0=mybir.AluOpType.mult,
                op1=mybir.AluOpType.mult,
            )
            nc.vector.tensor_copy(outs_acc[:, ti, :], idxu[:])

        # flush this block's outputs so only the last block's tail is exposed.
        # DRAM layout is [P, n_tiles*K] (token-tile-major per partition) so
        # each partition line is one contiguous run; host unshuffles for free.
        # flushed from the DVE queue: keeps the SP/HWDGE ring free for x, so
        # a flush waiting on routing can't head-of-line-block the next block's
        # x prefetches.
        ti0 = t0 // P
        nc.scalar.dma_start(
            wts_d.rearrange("p (t k) -> p t k", k=K)[:, ti0 : ti0 + n_sub, :],
            outw_acc[:, ti0 : ti0 + n_sub, :],
        )
        nc.scalar.dma_start(
            sel_d.rearrange("p (t k) -> p t k", k=K)[:, ti0 : ti0 + n_sub, :],
            outs_acc[:, ti0 : ti0 + n_sub, :],
        )


_NC_CACHE = {}


TAPER = True


def _build(mode=MODE, t_core=T_CORE, blk=BLK, repeat=1, taper=None):
    if taper is None:
        taper = TAPER
    key = (mode, t_core, blk, repeat, taper)
    if key in _NC_CACHE:
        return _NC_CACHE[key]
    nc = bacc.Bacc("TRN2", target_bir_lowering=False, debug=False)
    f32 = mybir.dt.float32
    f16 = mybir.dt.float16
    if mode == "f16x3":
        ins = [
            nc.dram_tensor("xh", [H, t_core], f16, kind="ExternalInput").ap(),
            nc.dram_tensor("xl", [H, t_core], f16, kind="ExternalInput").ap(),
            nc.dram_tensor("wh", [H, E], f16, kind="ExternalInput").ap(),
            nc.dram_tensor("wl", [H, E], f16, kind="ExternalInput").ap(),
            nc.dram_tensor("bias", [E], f32, kind="ExternalInput").ap(),
        ]
    else:
        mdt = mybir.dt.float32r if mode == "f32r" else f32
        ins = [
            nc.dram_tensor("xT", [H, t_core], mdt, kind="ExternalInput").ap(),
            nc.dram_tensor("w", [H, E], mdt, kind="ExternalInput").ap(),
            nc.dram_tensor("bias", [E], f32, kind="ExternalInput").ap(),
        ]
    n_tiles = t_core // P
    outs = [
        nc.dram_tensor("wts", [P, n_tiles * K], f32, kind="ExternalOutput").ap(),
        nc.dram_tensor(
            "sel", [P, n_tiles * K], mybir.dt.int32, kind="ExternalOutput"
        ).ap(),
    ]
    with tile.TileContext(nc) as tc:
        _gate_kernel(
            tc, outs, ins, T_core=t_core, BLK=blk, mode=mode, repeat=repeat,
            taper=taper,
        )
    nc.compile()
    _NC_CACHE[key] = nc
    return nc


def _make_in_maps(x, W_gate, bias, mode=MODE):
    x = np.asarray(x, dtype=np.float32)
    W_gate = np.asarray(W_gate, dtype=np.float32)
    bias = np.asarray(bias, dtype=np.float32)
    in_maps = []
    if mode == "f16x3":
        Wh = W_gate.astype(np.float16)
        Wl = (W_gate - Wh.astype(np.float32)).astype(np.float16)
        for c in range(N_CORES):
            xT = x[c * T_CORE : (c + 1) * T_CORE].T
            xh = np.ascontiguousarray(xT.astype(np.float16))
            xl = np.ascontiguousarray(
                (xT - xh.astype(np.float32)).astype(np.float16)
            )
            in_maps.append({"xh": xh, "xl": xl, "wh": Wh, "wl": Wl, "bias": bias})
    else:
        for c in range(N_CORES):
            xT = np.ascontiguousarray(x[c * T_CORE : (c + 1) * T_CORE].T)
            in_maps.append({"xT": xT, "w": W_gate, "bias": bias})
    return in_maps


_NEFF_CACHE_DIR = "/tmp/bass_neff_cache"
_neff_cache_installed = False


def _install_neff_cache():
    """Cache compiled NEFFs by BIR hash so repeat runs skip walrus."""
    global _neff_cache_installed
    if _neff_cache_installed:
        return
    import hashlib
    import os
    import shutil

    from concourse import bass2jax, bass_utils

    orig = bass_utils.compile_bir_kernel

    def cached(bir_json, tmpdir, neff_name="file.neff"):
        h = hashlib.sha256(bir_json).hexdigest()[:24]
        os.makedirs(_NEFF_CACHE_DIR, exist_ok=True)
        cpath = os.path.join(_NEFF_CACHE_DIR, h + ".neff")
        out = os.path.join(tmpdir, neff_name)
        if os.path.exists(cpath):
            shutil.copy(cpath, out)
            return out
        p = orig(bir_json, tmpdir, neff_name)
        try:
            shutil.copy(p, cpath)
        except OSError:
            pass
        return p

    bass2jax.compile_bir_kernel = cached
    _neff_cache_installed = True


def run_on_hw(x, W_gate, bias, mode=MODE, trace=False, **kwargs):
    from concourse import bass_utils

    _install_neff_cache()
    nc = _build(mode)
    in_maps = _make_in_maps(x, W_gate, bias, mode)
    res = bass_utils.run_bass_kernel_spmd(
        nc, in_maps, list(range(N_CORES)), trace=trace, **kwargs
    )

    def unshuffle(a):
        # device layout [P, n_tiles*K]: row p, tile t -> token t*P + p
        a = np.asarray(a).reshape(P, T_CORE // P, K)
        return np.ascontiguousarray(a.transpose(1, 0, 2).reshape(T_CORE, K))

    wts = np.concatenate([unshuffle(r["wts"]) for r in res.results], axis=0)
    sel = np.concatenate([unshuffle(r["sel"]) for r in res.results], axis=0)
    return (wts.astype(np.float32), sel.astype(np.int32)), res


def kernel(x, W_gate, bias):
    (wts, sel), _ = run_on_hw(x, W_gate, bias, MODE)
    return wts, sel



# revision 39
# speedup vs baseline: 1.2323x; 1.1582x over previous
"""DeepseekV3 MoE gate (moe_routing) for 8x TRN2 NeuronCores.

Sharding: data-parallel over tokens. Each core gets a 2048-token shard of x
(pre-transposed on host to [H, Tc] so both matmul operands DMA naturally with
the contraction dim on partitions); the small gate weight [7168, 256] and bias
are replicated.

Per-core pipeline, per 128-token tile:
  PE:   logits = xT_chunk.T @ W_chunk accumulated over 56 K-chunks in PSUM
  ACT:  s0 = sigmoid(logits)  (PSUM -> SBUF eviction fused)
  DVE:  b = s0 + bias; per-group Max8 -> top-2 sums -> group top-4 threshold
        -> additive mask; Max8/MaxIndex8 over masked scores -> top-8 experts;
        z = s0 + #{k: ms >= vals8[k]} (rank bands), Max8(z) recovers the
        selected s0 in exact rank order without any gather;
        normalize * 2.5, emit weights + indices.

Matmul runs as an fp16 hi/lo 3-term split (xh@Wh + xh@Wl + xl@Wh, split done
host-side) at 1 cycle/row on the PE with ~2e-7 logit error — fp32-grade
ranking fidelity at ~3.5x the speed of the PE's serialized fp32 mode.
"""

import sys

if "/opt/trn_rl_repo" not in sys.path:
    sys.path.insert(0, "/opt/trn_rl_repo")

from contextlib import ExitStack

import numpy as np

import concourse.bass as bass
import concourse.mybir as mybir
import concourse.tile as tile
from concourse import bacc
from concourse._compat import with_exitstack

H = 7168
E = 256
G = 8
EPG = E // G  # 32
K = 8
NEG = -1.0e30
ROUTE_SCALE = 2.5
P = 128

N_CORES = 8
T_FULL = 16384
T_CORE = T_FULL // N_CORES  # 2048

MODE = "f16x3"  # "f32r" | "f32" | "f16x3" | "fp8x3"
BLK = 512
KPACK_DEFAULT = 4
XBUFS = 8

# fp8x3 runs one 128-token tile per block: PSUM stops stagger every ~9us so
# routing drains continuously instead of bunching at block ends, and the
# tile-order x stream keeps DMA descriptors wide at any block size.
FP8_BLK = 128
FP8_KPACK = 8
FP8_XBUFS = 5


def _block_schedule(T_core, BLK, taper):
    """Token-block schedule. With taper=True the final blocks shrink
    (512->256->128->128) so the post-matmul routing tail drains one tile deep
    instead of four, each block's routing hiding under later blocks' DMA."""
    if not taper:
        return [(i * BLK, BLK) for i in range(T_core // BLK)]
    blocks = []
    t = 0
    rem = T_core
    while rem > 0:
        if rem > BLK:
            bs = BLK
        elif rem == BLK and BLK >= 4 * P:
            bs = BLK // 2
        elif rem > 2 * P:
            bs = rem - 2 * P
        else:
            bs = P
        bs = min(bs, rem)
        blocks.append((t, bs))
        t += bs
        rem -= bs
    return blocks


def _fp8_sched(n_blocks, n_groups):
    """Emission order of (block, k-group) units for fp8x3. MUST match the
    kernel's consumption order (block-major) — the x stream is laid out in
    exactly this order."""
    return [(b, g) for b in range(n_blocks) for g in range(n_groups)]


def np_algo_reference(x, W, bias):
    """Numpy mirror of the kernel algorithm (for validation in tests)."""
    x = x.astype(np.float32)
    T = x.shape[0]
    logits = (x.astype(np.float64) @ W.astype(np.float64)).astype(np.float32)
    s0 = (1.0 / (1.0 + np.exp(-logits.astype(np.float64)))).astype(np.float32)
    b = s0 + bias.astype(np.float32)
    bg = b.reshape(T, G, EPG)
    top2 = np.sort(bg, axis=-1)[:, :, -2:]
    gs = (top2[:, :, 0] + top2[:, :, 1]).astype(np.float32)
    gsort = np.sort(gs, axis=-1)[:, ::-1]
    thresh = gsort[:, 3:4]
    pen = np.where(gs >= thresh, np.float32(0.0), np.float32(NEG))
    ms = b + np.repeat(pen, EPG, axis=1)
    order = np.argsort(-ms, axis=-1, kind="stable")[:, :K]
    s0sel = np.take_along_axis(s0, order, axis=-1)
    q = np.arange(K, 0, -1).astype(np.float32)
    z = (q[None, :] + s0sel).astype(np.float32)
    s0sel_rt = (z - q[None, :]).astype(np.float32)
    ssum = s0sel_rt.sum(-1, keepdims=True, dtype=np.float32)
    wts = (s0sel_rt * ((np.float32(1.0) / ssum) * np.float32(ROUTE_SCALE))).astype(
        np.float32
    )
    return wts, order.astype(np.int32)


@with_exitstack
def _gate_kernel(
    ctx: ExitStack,
    tc: tile.TileContext,
    outs,
    ins,
    T_core: int,
    BLK: int = 512,
    mode: str = "f32r",
    repeat: int = 1,
    taper: bool = False,
):
    nc = tc.nc
    if len(outs) == 3:
        wts_d, sel_d, lgd_d = outs
    else:
        wts_d, sel_d = outs
        lgd_d = None
    if mode == "f16x3":
        xh_d, xl_d, wh_d, wl_d, bias_d = ins
    elif mode == "fp8x3":
        xs_d, w_d, bias_d = ins
        xs_off = 0
    else:
        xT_d, w_d, bias_d = ins

    n_k = H // P  # 56
    if mode == "fp8x3":
        BLK = FP8_BLK
    KPACK = FP8_KPACK if mode == "fp8x3" else KPACK_DEFAULT
    assert n_k % KPACK == 0
    n_tiles = T_core // P
    blocks = _block_schedule(T_core, BLK, taper)

    f32 = mybir.dt.float32
    f16 = mybir.dt.float16
    assert T_core % BLK == 0 and BLK % P == 0

    const = ctx.enter_context(tc.tile_pool(name="const", bufs=1))
    wpool = ctx.enter_context(tc.tile_pool(name="wpool", bufs=1))
    xpool = ctx.enter_context(
        tc.tile_pool(name="xpool", bufs=FP8_XBUFS if mode == "fp8x3" else XBUFS)
    )
    if mode == "fp8x3":
        # PSUM is bank-granular (2KB/partition): pack p0|p6 per subtile into
        # one bank, p12 for subtile pairs into another -> 6 banks per block,
        # 8 total for cross-block overlap.
        ppoolA = ctx.enter_context(tc.tile_pool(name="ppoolA", bufs=5, space="PSUM"))
        ppoolB = ctx.enter_context(tc.tile_pool(name="ppoolB", bufs=3, space="PSUM"))
    else:
        ppool = ctx.enter_context(tc.tile_pool(name="ppool", bufs=8, space="PSUM"))
    spool = ctx.enter_context(tc.tile_pool(name="spool", bufs=3))
    # s0 lives from a block's eviction head until its lag-1 routing: 2 blocks
    # of subtiles + margin
    s0pool = ctx.enter_context(tc.tile_pool(name="s0pool", bufs=9))
    opool = ctx.enter_context(tc.tile_pool(name="opool", bufs=3))
    pending_routing = []

    # ---- constants ----
    # bias rides the DVE queue so the x stream owns the SP/HWDGE ring
    bias_bc = const.tile([P, E], f32)
    nc.scalar.dma_start(bias_bc[:], bias_d.unsqueeze(0).to_broadcast([P, E]))

    qrow32 = const.tile([P, K], f32)
    for k in range(K):
        nc.vector.memset(qrow32[:, k : k + 1], float(K - k))

    # output accumulators: one SBUF row-block per 128-token tile, DMA'd once
    outw_acc = const.tile([P, n_tiles, K], f32)
    outs_acc = const.tile([P, n_tiles, K], mybir.dt.int32)

    # ---- resident weights ----
    # Streamed on the HWDGE queue in KPACK-sized pieces, issued interleaved
    # with block 0's x stream below so the PE starts after one piece of each
    # instead of after the whole 7MB W load.
    if mode == "f16x3":
        w_all_h = wpool.tile([P, n_k, E], f16)
        w_all_l = wpool.tile([P, n_k, E], f16)
        wh_view = wh_d.rearrange("(k p) e -> p k e", p=P)
        wl_view = wl_d.rearrange("(k p) e -> p k e", p=P)

        def load_w_piece(k0, k1):
            nc.sync.dma_start(w_all_h[:, k0:k1, :], wh_view[:, k0:k1, :])
            nc.sync.dma_start(w_all_l[:, k0:k1, :], wl_view[:, k0:k1, :])
    elif mode == "fp8x3":
        # resident W levels, plane-interleaved (w0, w1, w2) per k-chunk so any
        # adjacent level pair is one strided AP. DRAM is pre-shuffled to the
        # SBUF layout [P, n_k*3*E] so each partition line is one contiguous run.
        f8 = mybir.dt.float8e4
        w_all = wpool.tile([P, n_k, 3, E], f8)
        w_flat = w_all[:].rearrange("p k three e -> p k (three e)")
        w_view = w_d.rearrange("p (k fe) -> p k fe", fe=3 * E)

        def load_w_piece(k0, k1):
            nc.sync.dma_start(w_flat[:, k0:k1, :], w_view[:, k0:k1, :])
    else:
        mdt = mybir.dt.float32r if mode == "f32r" else f32
        w_all = wpool.tile([P, n_k, E], mdt)
        w_view = w_d.rearrange("(k p) e -> p k e", p=P)

        def load_w_piece(k0, k1):
            nc.sync.dma_start(w_all[:, k0:k1, :], w_view[:, k0:k1, :])

    w_loaded = 0  # k-chunks of W issued so far

    # ---- main loop ----
    for rep, (tb, (t0, bs)) in [
        (r, b) for r in range(repeat) for b in enumerate(blocks)
    ]:
        n_sub = bs // P
        if mode == "fp8x3":
            pas = [
                ppoolA.tile([P, 2, E], f32, name=f"pa_{rep}_{tb}_{s}", tag="pa")
                for s in range(n_sub)
            ]
            pbs = [
                ppoolB.tile([P, 2, E], f32, name=f"pb_{rep}_{tb}_{j}", tag="pb")
                for j in range((n_sub + 1) // 2)
            ]
            psums3 = [
                (pas[s][:, 0, :], pas[s][:, 1, :], pbs[s // 2][:, s % 2, :])
                for s in range(n_sub)
            ]
        else:
            psums = []
            for s in range(n_sub):
                pt = ppool.tile([P, E], f32, name=f"psum_{rep}_{tb}_{s}", tag="psum")
                psums.append(pt)

        for k0 in range(0, n_k, KPACK):
            if w_loaded < n_k:
                # lazily stream the next W piece just ahead of its first use
                load_w_piece(w_loaded, min(w_loaded + KPACK, n_k))
                w_loaded = min(w_loaded + KPACK, n_k)
            if mode == "fp8x3":
                f8 = mybir.dt.float8e4
                DR = mybir.MatmulPerfMode.DoubleRow
                glen = KPACK * 3 * bs
                xs = xpool.tile([P, KPACK, 3, bs], f8, tag=f"xs{bs}")
                nc.sync.dma_start(
                    xs[:].rearrange("p kk lev t -> p (kk lev t)"),
                    xs_d[xs_off : xs_off + P * glen].rearrange(
                        "(p f) -> p f", p=P
                    ),
                )
                xs_off += P * glen
                # 6 level-products per chunk as 3 DoubleRow matmuls:
                #   P6  += x1[k]@w0[k] + x0[k]@w1[k]     (planes within chunk)
                #   P12 += x1[k]@w1[k] + x0[k]@w2[k]     (planes within chunk)
                #   P0  += x0[k]@w0[k] + x0[k+1]@w0[k+1] (chunk-paired)
                #   P12 += x2[k]@w0[k] + x2[k+1]@w0[k+1] (chunk-paired)
                # xs planes are (x1, x0, x2); w planes are (w0, w1, w2).
                for kk in range(KPACK):
                    k = k0 + kk
                    for s in range(n_sub):
                        ts_ = slice(s * P, (s + 1) * P)
                        p0, p6, p12 = psums3[s]
                        # p0|p6 share one PSUM bank; 'start' zeroes the whole
                        # 2KB zero region, so the bank gets exactly one start
                        # (p6's k=0) and one stop (p0's last pair).
                        nc.tensor.matmul(
                            p6,
                            xs[:, kk, 0:2, ts_],
                            w_all[:, k, 0:2, :],
                            start=(k == 0),
                            stop=False,
                            perf_mode=DR,
                        )
                        nc.tensor.matmul(
                            p12,
                            xs[:, kk, 0:2, ts_],
                            w_all[:, k, 1:3, :],
                            start=(k == 0),
                            stop=False,
                            perf_mode=DR,
                        )
                        if k % 2 == 1:
                            nc.tensor.matmul(
                                p0,
                                xs[:, kk - 1 : kk + 1, 1, ts_],
                                w_all[:, k - 1 : k + 1, 0, :],
                                start=False,
                                stop=(k == n_k - 1),
                                perf_mode=DR,
                            )
                            nc.tensor.matmul(
                                p12,
                                xs[:, kk - 1 : kk + 1, 2, ts_],
                                w_all[:, k - 1 : k + 1, 0, :],
                                start=False,
                                stop=(k == n_k - 1),
                                perf_mode=DR,
                            )
            elif mode == "f16x3":
                xch = xpool.tile([P, KPACK, bs], f16, tag="xch")
                xcl = xpool.tile([P, KPACK, bs], f16, tag="xcl")
                nc.sync.dma_start(
                    xch[:],
                    xh_d[k0 * P : (k0 + KPACK) * P, t0 : t0 + bs].rearrange(
                        "(kk p) t -> p kk t", p=P
                    ),
                )
                nc.sync.dma_start(
                    xcl[:],
                    xl_d[k0 * P : (k0 + KPACK) * P, t0 : t0 + bs].rearrange(
                        "(kk p) t -> p kk t", p=P
                    ),
                )
                for kk in range(KPACK):
                    k = k0 + kk
                    start = k == 0
                    stop = k == n_k - 1
                    for s in range(n_sub):
                        lh = xch[:, kk, s * P : (s + 1) * P]
                        ll = xcl[:, kk, s * P : (s + 1) * P]
                        # xh stationary twice in a row -> cheaper weight reload
                        nc.tensor.matmul(
                            psums[s][:], lh, w_all_h[:, k, :], start=start, stop=False
                        )
                        nc.tensor.matmul(
                            psums[s][:], lh, w_all_l[:, k, :], start=False, stop=False
                        )
                        nc.tensor.matmul(
                            psums[s][:], ll, w_all_h[:, k, :], start=False, stop=stop
                        )
            else:
                xc = xpool.tile([P, KPACK, bs], mdt, tag="xch")
                nc.sync.dma_start(
                    xc[:],
                    xT_d[k0 * P : (k0 + KPACK) * P, t0 : t0 + bs].rearrange(
                        "(kk p) t -> p kk t", p=P
                    ),
                )
                for kk in range(KPACK):
                    k = k0 + kk
                    for s in range(n_sub):
                        nc.tensor.matmul(
                            psums[s][:],
                            xc[:, kk, s * P : (s + 1) * P],
                            w_all[:, k, :],
                            start=(k == 0),
                            stop=(k == n_k - 1),
                        )

        # Head: combine PSUMs + sigmoid immediately so banks free promptly.
        # The routing chains for this block are emitted AFTER the next
        # block's matmuls (lag-1), so a previous block's ~6us/tile routing
        # can't head-of-line-block the next block's combines on the DVE FIFO
        # (which would hold PSUM banks and stall the PE).
        s0_tiles = []
        for s in range(n_sub):
            s0 = s0pool.tile([P, E], f32, tag="s0")
            if mode == "fp8x3":
                # logits*2^6 = P0 + P6*2^-6 + P12*2^-12; the final 2^-6 rides
                # the sigmoid's input scale on the ACT engine. A DVE op may
                # read only ONE PSUM input, so the combine is three ops each
                # touching a single PSUM bank.
                p0, p6, p12 = psums3[s]
                lg = spool.tile([P, E], f32, tag="lg")
                nc.vector.tensor_scalar(
                    lg[:], p12, 2.0**-6, None, op0=mybir.AluOpType.mult
                )
                nc.vector.tensor_add(lg[:], lg[:], p6)
                nc.vector.scalar_tensor_tensor(
                    lg[:],
                    lg[:],
                    2.0**-6,
                    p0,
                    op0=mybir.AluOpType.mult,
                    op1=mybir.AluOpType.add,
                )
                nc.scalar.activation(
                    s0[:],
                    lg[:],
                    mybir.ActivationFunctionType.Sigmoid,
                    scale=2.0**-6,
                )
                if lgd_d is not None:
                    ti_dbg = (t0 + s * P) // P
                    nc.scalar.dma_start(
                        lgd_d.rearrange("p (t e) -> p t e", e=E)[:, ti_dbg, :],
                        lg[:],
                    )
            else:
                nc.scalar.activation(
                    s0[:], psums[s][:], mybir.ActivationFunctionType.Sigmoid
                )
            s0_tiles.append(s0)

        def _routing(t0=t0, bs=bs, n_sub=n_sub, s0_tiles=s0_tiles):
            for s in range(n_sub):
                trow = t0 + s * P
                s0 = s0_tiles[s]
                b = spool.tile([P, E], f32, tag="b")
                nc.vector.tensor_add(b[:], s0[:], bias_bc[:])
                gmax = opool.tile([P, G * 8], f32, tag="gmax")
                for g in range(G):
                    nc.vector.max(
                        out=gmax[:, g * 8 : (g + 1) * 8],
                        in_=b[:, g * EPG : (g + 1) * EPG],
                    )
                gv = gmax[:].rearrange("p (g c) -> p g c", g=G)
                gs = opool.tile([P, G], f32, tag="gs")
                nc.vector.tensor_add(gs[:], gv[:, :, 0], gv[:, :, 1])
                gtop = opool.tile([P, 8], f32, tag="gtop")
                nc.vector.max(out=gtop[:], in_=gs[:])
                pen = opool.tile([P, G], f32, tag="pen")
                nc.vector.tensor_scalar(
                    pen[:],
                    gs[:],
                    gtop[:, 3:4],
                    None,
                    op0=mybir.AluOpType.is_ge,
                )
                nc.vector.tensor_scalar(
                    pen[:],
                    pen[:],
                    1.0,
                    -NEG,
                    op0=mybir.AluOpType.subtract,
                    op1=mybir.AluOpType.mult,
                )
                ms = spool.tile([P, E], f32, tag="ms")
                pen_bc = pen[:].unsqueeze(2).to_broadcast([P, G, EPG])
                nc.vector.tensor_add(
                    ms[:].rearrange("p (g c) -> p g c", g=G),
                    b[:].rearrange("p (g c) -> p g c", g=G),
                    pen_bc,
                )
                vals8 = opool.tile([P, K], f32, tag="vals8")
                nc.vector.max(out=vals8[:], in_=ms[:])
                idxu = opool.tile([P, K], mybir.dt.uint16, tag="idxu")
                nc.vector.max_index(idxu[:], vals8[:], ms[:])
                # z[p,e] = s0[p,e] + #{k : ms[p,e] >= vals8[p,k]}
                # selected rank-r expert lands in band (8-r, 9-r); rest in (0,1)
                z = spool.tile([P, E], f32, tag="z")
                nc.vector.scalar_tensor_tensor(
                    z[:],
                    ms[:],
                    vals8[:, 0:1],
                    s0[:],
                    op0=mybir.AluOpType.is_ge,
                    op1=mybir.AluOpType.add,
                )
                for k in range(1, K):
                    nc.vector.scalar_tensor_tensor(
                        z[:],
                        ms[:],
                        vals8[:, k : k + 1],
                        z[:],
                        op0=mybir.AluOpType.is_ge,
                        op1=mybir.AluOpType.add,
                    )
                zv = opool.tile([P, K], f32, tag="zv")
                nc.vector.max(out=zv[:], in_=z[:])
                s0sel = opool.tile([P, K], f32, tag="s0sel")
                nc.vector.tensor_sub(s0sel[:], zv[:], qrow32[:])
                ssum = opool.tile([P, 1], f32, tag="ssum")
                nc.vector.tensor_reduce(
                    ssum[:], s0sel[:], axis=mybir.AxisListType.X, op=mybir.AluOpType.add
                )
                rec = opool.tile([P, 1], f32, tag="rec")
                nc.vector.reciprocal(rec[:], ssum[:])
                ti = trow // P
                nc.vector.tensor_scalar(
                    outw_acc[:, ti, :],
                    s0sel[:],
                    rec[:],
                    ROUTE_SCALE,
                    op0=mybir.AluOpType.mult,
                    op1=mybir.AluOpType.mult,
                )
                nc.vector.tensor_copy(outs_acc[:, ti, :], idxu[:])

            # flush this block's outputs (ACT HWDGE queue: keeps the SP ring
            # free for x, so a flush waiting on routing can't head-of-line
            # block the next block's x prefetches)
            ti0 = t0 // P
            nc.scalar.dma_start(
                wts_d.rearrange("p (t k) -> p t k", k=K)[:, ti0 : ti0 + n_sub, :],
                outw_acc[:, ti0 : ti0 + n_sub, :],
            )
            nc.scalar.dma_start(
                sel_d.rearrange("p (t k) -> p t k", k=K)[:, ti0 : ti0 + n_sub, :],
                outs_acc[:, ti0 : ti0 + n_sub, :],
            )

        pending_routing.append(_routing)
        if len(pending_routing) > 1:
            pending_routing.pop(0)()

    for fn in pending_routing:
        fn()


_NC_CACHE = {}


TAPER = True


def _build(mode=MODE, t_core=T_CORE, blk=BLK, repeat=1, taper=None, dbg=False):
    if taper is None:
        taper = TAPER
    key = (mode, t_core, blk, repeat, taper, dbg)
    if key in _NC_CACHE:
        return _NC_CACHE[key]
    nc = bacc.Bacc("TRN2", target_bir_lowering=False, debug=False)
    f32 = mybir.dt.float32
    f16 = mybir.dt.float16
    if mode == "f16x3":
        ins = [
            nc.dram_tensor("xh", [H, t_core], f16, kind="ExternalInput").ap(),
            nc.dram_tensor("xl", [H, t_core], f16, kind="ExternalInput").ap(),
            nc.dram_tensor("wh", [H, E], f16, kind="ExternalInput").ap(),
            nc.dram_tensor("wl", [H, E], f16, kind="ExternalInput").ap(),
            nc.dram_tensor("bias", [E], f32, kind="ExternalInput").ap(),
        ]
    elif mode == "fp8x3":
        f8 = mybir.dt.float8e4
        n_k = H // P
        ins = [
            nc.dram_tensor("xs", [3 * H * t_core], f8, kind="ExternalInput").ap(),
            nc.dram_tensor("w", [P, n_k * 3 * E], f8, kind="ExternalInput").ap(),
            nc.dram_tensor("bias", [E], f32, kind="ExternalInput").ap(),
        ]
    else:
        mdt = mybir.dt.float32r if mode == "f32r" else f32
        ins = [
            nc.dram_tensor("xT", [H, t_core], mdt, kind="ExternalInput").ap(),
            nc.dram_tensor("w", [H, E], mdt, kind="ExternalInput").ap(),
            nc.dram_tensor("bias", [E], f32, kind="ExternalInput").ap(),
        ]
    n_tiles = t_core // P
    outs = [
        nc.dram_tensor("wts", [P, n_tiles * K], f32, kind="ExternalOutput").ap(),
        nc.dram_tensor(
            "sel", [P, n_tiles * K], mybir.dt.int32, kind="ExternalOutput"
        ).ap(),
    ]
    if dbg:
        outs.append(
            nc.dram_tensor(
                "lgd", [P, n_tiles * E], f32, kind="ExternalOutput"
            ).ap()
        )
    with tile.TileContext(nc) as tc:
        _gate_kernel(
            tc, outs, ins, T_core=t_core, BLK=blk, mode=mode, repeat=repeat,
            taper=taper,
        )
    nc.compile()
    _NC_CACHE[key] = nc
    return nc


def _make_in_maps(x, W_gate, bias, mode=MODE):
    x = np.asarray(x, dtype=np.float32)
    W_gate = np.asarray(W_gate, dtype=np.float32)
    bias = np.asarray(bias, dtype=np.float32)
    in_maps = []
    if mode == "fp8x3":
        import ml_dtypes

        E4 = ml_dtypes.float8_e4m3

        def lev(a, scale):
            q = (a * scale).astype(E4)
            return q, a - q.astype(np.float32) / scale

        # three e4m3 levels per operand; W pre-scaled by 2^6 so its levels
        # stay in e4m3's normal range. logits recombine on-chip as
        # (P0 + P6*2^-6 + P12*2^-12) * 2^-6.
        V = W_gate * np.float32(2.0**6)
        w0, s1 = lev(V, 1.0)
        w1, s2 = lev(s1, 2.0**6)
        w2, _ = lev(s2, 2.0**12)
        n_k = H // P
        # resident W pre-shuffled to SBUF layout [P, n_k*3*E]
        wAll = np.stack([w0, w1, w2], axis=1).reshape(n_k, P, 3, E)
        wAll = np.ascontiguousarray(wAll.transpose(1, 0, 2, 3)).reshape(P, -1)
        blocks = _block_schedule(T_CORE, FP8_BLK, TAPER)
        KP = FP8_KPACK
        for c in range(N_CORES):
            xT = np.ascontiguousarray(x[c * T_CORE : (c + 1) * T_CORE].T)
            x0, r1 = lev(xT, 1.0)
            x1, r2 = lev(r1, 2.0**6)
            x2, _ = lev(r2, 2.0**12)
            # x stream in exact SBUF consumption order:
            # per (block, k-group): [P, KPACK, 3(levels x1,x0,x2), bs]
            xl = np.stack(
                [a.reshape(n_k, P, T_CORE) for a in (x1, x0, x2)], axis=0
            )  # [3, n_k, P, Tc]
            pieces = []
            for b, g in _fp8_sched(len(blocks), n_k // KP):
                t0, bs = blocks[b]
                blk = xl[:, g * KP : (g + 1) * KP, :, t0 : t0 + bs]
                pieces.append(
                    np.ascontiguousarray(blk.transpose(2, 1, 0, 3)).reshape(-1)
                )
            xs = np.concatenate(pieces)
            in_maps.append({"xs": xs, "w": wAll, "bias": bias})
    elif mode == "f16x3":
        Wh = W_gate.astype(np.float16)
        Wl = (W_gate - Wh.astype(np.float32)).astype(np.float16)
        for c in range(N_CORES):
            xT = x[c * T_CORE : (c + 1) * T_CORE].T
            xh = np.ascontiguousarray(xT.astype(np.float16))
            xl = np.ascontiguousarray(
                (xT - xh.astype(np.float32)).astype(np.float16)
            )
            in_maps.append({"xh": xh, "xl": xl, "wh": Wh, "wl": Wl, "bias": bias})
    else:
        for c in range(N_CORES):
            xT = np.ascontiguousarray(x[c * T_CORE : (c + 1) * T_CORE].T)
            in_maps.append({"xT": xT, "w": W_gate, "bias": bias})
    return in_maps


_NEFF_CACHE_DIR = "/tmp/bass_neff_cache"
_neff_cache_installed = False


def _install_neff_cache():
    """Cache compiled NEFFs by BIR hash so repeat runs skip walrus."""
    global _neff_cache_installed
    if _neff_cache_installed:
        return
    import hashlib
    import os
    import shutil

    from concourse import bass2jax, bass_utils

    orig = bass_utils.compile_bir_kernel

    def cached(bir_json, tmpdir, neff_name="file.neff"):
        h = hashlib.sha256(bir_json).hexdigest()[:24]
        os.makedirs(_NEFF_CACHE_DIR, exist_ok=True)
        cpath = os.path.join(_NEFF_CACHE_DIR, h + ".neff")
        out = os.path.join(tmpdir, neff_name)
        if os.path.exists(cpath):
            shutil.copy(cpath, out)
            return out
        p = orig(bir_json, tmpdir, neff_name)
        try:
            shutil.copy(p, cpath)
        except OSError:
            pass
        return p

    bass2jax.compile_bir_kernel = cached
    _neff_cache_installed = True


def run_on_hw(x, W_gate, bias, mode=MODE, trace=False, **kwargs):
    from concourse import bass_utils

    _install_neff_cache()
    nc = _build(mode)
    in_maps = _make_in_maps(x, W_gate, bias, mode)
    res = bass_utils.run_bass_kernel_spmd(
        nc, in_maps, list(range(N_CORES)), trace=trace, **kwargs
    )

    def unshuffle(a):
        # device layout [P, n_tiles*K]: row p, tile t -> token t*P + p
        a = np.asarray(a).reshape(P, T_CORE // P, K)
        return np.ascontiguousarray(a.transpose(1, 0, 2).reshape(T_CORE, K))

    wts = np.concatenate([unshuffle(r["wts"]) for r in res.results], axis=0)
    sel = np.concatenate([unshuffle(r["sel"]) for r in res.results], axis=0)
    return (wts.astype(np.float32), sel.astype(np.int32)), res


def kernel(x, W_gate, bias):
    (wts, sel), _ = run_on_hw(x, W_gate, bias, MODE)
    return wts, sel

